# revision 32
# baseline (speedup 1.0000x reference)
"""AttentionBlock kernel for Trainium2, data-parallel over 8 NeuronCores.

Problem: x[16,256,32,32]; per sample (S=1024 tokens, C=256 channels):
  xs = x.reshape(C, S).T                      # [S, C]
  qkv = xs @ w_qkv + b_qkv                    # [S, 768] -> heads q,k,v (4 x 64)
  attn = softmax(q k^T / 8) ; o = attn @ v    # per head
  out = (concat_h o) @ w_out + b_out + xs     # [S, C] -> emitted as [C, S]

Device strategy (per core, 2 samples):
  Everything is computed in the TRANSPOSED layout so no on-chip transposes
  are needed:
    qkT[n, i]  = w_qkv[:, n].T @ x_nat          (x natural [C, S] == xs^T)
    S^T[j, i]  = K^T.T-contraction: lhsT=K^T[d, j], rhs=Q^T[d, i]
    P^T        = exp(S^T / 8)  (softmax w/o max-subtract; scores ~ N(0,1))
    O^T[d+1,i] = accum_j  V_ext[j, d|1].T @ P^T[j, i]  (ones col -> row sums)
    out^T[c,i] = w_out[m, c].T @ (O^T normalized)  + b_eff + x_nat
  b_qkv(v-part) commutes through softmax (rows sum to 1) so it is folded on
  the host into b_eff = b_out + b_v @ w_out.
"""

import numpy as np
from contextlib import ExitStack

B, C, S = 16, 256, 1024
NH, DK = 4, 64
NCORES = 8
BS = B // NCORES  # samples per core

_CACHE = {}


def _build_program(tc, xd, wqk, wv, wo, bqk, beff, yd):
    """Emit the per-core program into TileContext tc.

    xd   : [BS, 256, 1024] f32   input sample block (natural [C, S] layout)
    wqk  : [256, 512] bf16       qkv weights, columns permuted to [q0..q3 | k0..k3]
    wv   : [256, 256] bf16       v weights, columns [v0..v3]
    wo   : [256, 256] bf16       output projection
    bqk  : [128, 4] f32          q|k bias, bqk[p, t] = b[t*128 + p]
    beff : [128, 2] f32          b_out + b_v @ w_out, beff[p, ct] = b[ct*128 + p]
    yd   : [BS, 256, 1024] f32   output (natural [C, S] layout)
    """
    import concourse.bass as bass
    from concourse import mybir

    nc = tc.nc
    F32 = mybir.dt.float32
    BF16 = mybir.dt.bfloat16
    EXP = mybir.ActivationFunctionType.Exp
    IDENT = mybir.ActivationFunctionType.Identity
    MULT = mybir.AluOpType.mult
    ADD = mybir.AluOpType.add

    ctx = ExitStack()
    wts = ctx.enter_context(tc.tile_pool(name="wts", bufs=1))
    xfp = ctx.enter_context(tc.tile_pool(name="xfp", bufs=2))
    xbp = ctx.enter_context(tc.tile_pool(name="xbp", bufs=2))
    qkp = ctx.enter_context(tc.tile_pool(name="qkp", bufs=2))
    vvp = ctx.enter_context(tc.tile_pool(name="vvp", bufs=2))
    ptp = ctx.enter_context(tc.tile_pool(name="ptp", bufs=4))
    rrp = ctx.enter_context(tc.tile_pool(name="rrp", bufs=2))
    rbp = ctx.enter_context(tc.tile_pool(name="rbp", bufs=2))
    orp = ctx.enter_context(tc.tile_pool(name="orp", bufs=5))
    drp = ctx.enter_context(tc.tile_pool(name="drp", bufs=2, space="DRAM"))
    otp = ctx.enter_context(tc.tile_pool(name="otp", bufs=2))
    yop = ctx.enter_context(tc.tile_pool(name="yop", bufs=3))
    psb = ctx.enter_context(tc.tile_pool(name="psb", bufs=2, space="PSUM"))
    pso = ctx.enter_context(tc.tile_pool(name="pso", bufs=2, space="PSUM"))

    # --- weights / biases, loaded once ---
    wqk_sb = wts.tile([128, 2, 512], BF16)
    nc.sync.dma_start(wqk_sb[:], wqk[:, :].rearrange("(kc p) n -> p kc n", p=128))
    wv_sb = wts.tile([128, 2, 256], BF16)
    nc.sync.dma_start(wv_sb[:], wv[:, :].rearrange("(kc p) n -> p kc n", p=128))
    wo_sb = wts.tile([128, 2, 256], BF16)
    nc.sync.dma_start(wo_sb[:], wo[:, :].rearrange("(kc p) n -> p kc n", p=128))
    # Biases are staged through a DVE copy so downstream DVE consumers get a
    # same-engine dependency (no extra semaphore wait slot on the consumer —
    # walrus limits wait commands per instruction).
    bqk_st = wts.tile([128, 4], F32)
    nc.sync.dma_start(bqk_st[:], bqk[:, :])
    bqk_sb = wts.tile([128, 4], F32)
    nc.vector.tensor_copy(bqk_sb[:], bqk_st[:])
    beff_st = wts.tile([128, 2], F32)
    nc.sync.dma_start(beff_st[:], beff[:, :])
    beff_sb = wts.tile([128, 2], F32)
    nc.vector.tensor_copy(beff_sb[:], beff_st[:])

    def phase_load(s, st):
        """x load + bf16 cast (hoisted to the program start for all samples
        so the inter-sample pipeline never waits on the input DMA)"""
        x_sb = xfp.tile([128, 2, 1024], F32, name=f"x_{s}", tag="x")
        nc.sync.dma_start(x_sb[:], xd[s].rearrange("(kc p) i -> p kc i", p=128))
        xb_sb = xbp.tile([128, 2, 1024], BF16, name=f"xb_{s}", tag="xb")
        nc.vector.tensor_copy(xb_sb[:], x_sb[:])
        st.update(x_sb=x_sb, xb_sb=xb_sb)

    def phase_a(s, st):
        """qkT/V projections"""
        x_sb, xb_sb = st["x_sb"], st["xb_sb"]
        qk_sb = qkp.tile([128, 4, 1024], BF16, name=f"qk_{s}", tag="qk")
        v_sb = vvp.tile([128, 8, 4, 65], BF16, name=f"v_{s}", tag="v")

        def qk_tile(t):
            ps_qk = psb.tile([128, 1024], F32, tag="big", name=f"psqk_{s}_{t}")
            for kc in range(2):
                for ns in range(2):
                    nc.tensor.matmul(
                        ps_qk[:, ns * 512:(ns + 1) * 512],
                        lhsT=wqk_sb[:, kc, t * 128:(t + 1) * 128],
                        rhs=xb_sb[:, kc, ns * 512:(ns + 1) * 512],
                        start=(kc == 0), stop=(kc == 1),
                    )
            nc.scalar.activation(qk_sb[:, t, :], ps_qk[:], IDENT,
                                 bias=bqk_sb[:, t:t + 1])

        qk_tile(0)
        qk_tile(2)
        nc.vector.memset(v_sb[:, :, :, 64:65], 1.0)
        for it in range(8):
            ps_v = psb.tile([128, 1024], F32, tag="big", name=f"psv_{s}_{it}")
            pv = ps_v[:, 0:256].rearrange("p (g d) -> p g d", g=4)
            for kc in range(2):
                nc.tensor.matmul(
                    pv,
                    lhsT=xb_sb[:, kc, it * 128:(it + 1) * 128],
                    rhs=wv_sb[:, kc, :],
                    start=(kc == 0), stop=(kc == 1),
                )
            nc.scalar.copy(v_sb[:, it, :, 0:64], pv)
        qk_tile(1)
        qk_tile(3)
        st.update(qk_sb=qk_sb, v_sb=v_sb)

    def phase_b(s, st):
        """attention: heads sequential; pss bufs=2 gives one-jc lookahead for
        the exp/scores overlap; po bufs=2 overlaps adjacent heads; softmax
        normalization (reciprocal + gpsimd partition broadcast + multiply)
        is inlined per head and hides under the next head's attention."""
        qk_sb, v_sb = st["qk_sb"], st["v_sb"]
        ot_sb = otp.tile([128, 2, 1024], BF16, name=f"ot_{s}", tag="ot")
        for h in range(4):
            bp = 64 * (h % 2)
            po = pso.tile([128, 1024], F32, tag="po", name=f"pso_{s}_{h}")
            for jc in range(8):
                pss = psb.tile([128, 1024], F32, tag="big", name=f"pss_{s}_{h}_{jc}")
                for ns in range(2):
                    nc.tensor.matmul(
                        pss[:, ns * 512:(ns + 1) * 512],
                        lhsT=qk_sb[bp:bp + 64, 2 + h // 2, jc * 128:(jc + 1) * 128],
                        rhs=qk_sb[bp:bp + 64, h // 2, ns * 512:(ns + 1) * 512],
                        start=True, stop=True,
                    )
                pt_sb = ptp.tile([128, 1024], BF16, tag="pt",
                                 name=f"pt_{s}_{h}_{jc}")
                nc.scalar.activation(pt_sb[:], pss[:], EXP,
                                     scale=float(DK) ** -0.5)
                for ns in range(2):
                    nc.tensor.matmul(
                        po[0:65, ns * 512:(ns + 1) * 512],
                        lhsT=v_sb[:, jc, h, :],
                        rhs=pt_sb[:, ns * 512:(ns + 1) * 512],
                        start=(jc == 0), stop=(jc == 7),
                    )
            # rows 0:64 = unnormalized P@V, row 64 = softmax denominator
            rrow = rrp.tile([1, 1024], F32, tag="rr", name=f"rr_{s}_{h}")
            nc.vector.reciprocal(rrow[:], po[64:65, :])
            rb = rbp.tile([64, 1024], F32, tag="rb", name=f"rb_{s}_{h}")
            nc.gpsimd.partition_broadcast(rb[:], rrow[:], channels=64)
            nc.vector.tensor_tensor(
                ot_sb[bp:bp + 64, h // 2, :], po[0:64, :], rb[:], MULT)
        st.update(ot_sb=ot_sb)

    def phase_out(s, st):
        """output projection, bias+residual, store."""
        x_sb, ot_sb = st["x_sb"], st["ot_sb"]
        for ct in range(2):
            ps_r = pso.tile([128, 1024], F32, tag="po", name=f"psr_{s}_{ct}")
            for mc in range(2):
                for ns in range(2):
                    nc.tensor.matmul(
                        ps_r[:, ns * 512:(ns + 1) * 512],
                        lhsT=wo_sb[:, mc, ct * 128:(ct + 1) * 128],
                        rhs=ot_sb[:, mc, ns * 512:(ns + 1) * 512],
                        start=(mc == 0), stop=(mc == 1),
                    )
            y_sb = yop.tile([128, 1024], F32, tag="yo", name=f"y_{s}_{ct}")
            nc.vector.scalar_tensor_tensor(
                y_sb[:], ps_r[:], beff_sb[:, ct:ct + 1], x_sb[:, ct, :],
                ADD, ADD)
            nc.sync.dma_start(yd[s, ct * 128:(ct + 1) * 128, :], y_sb[:])

    # software-pipelined emission: per-pair softmax normalization is emitted
    # inside phase_b (it executes during the other pair's attention); sample
    # s+1's projections are emitted before sample s's output projection
    states = [{} for _ in range(BS)]
    for s in range(BS):
        phase_load(s, states[s])
    for s in range(BS):
        phase_a(s, states[s])
        phase_b(s, states[s])
        phase_out(s, states[s])

    return ctx


def _build_nc():
    import concourse.tile as tile
    from concourse import bacc, mybir

    nc = bacc.Bacc(trn_type="TRN2")
    F32, BF16 = mybir.dt.float32, mybir.dt.bfloat16
    xd = nc.dram_tensor("x", [BS, C, S], F32, kind="ExternalInput")
    wqk = nc.dram_tensor("wqk", [C, 2 * NH * DK], BF16, kind="ExternalInput")
    wv = nc.dram_tensor("wv", [C, NH * DK], BF16, kind="ExternalInput")
    wo = nc.dram_tensor("wo", [C, C], BF16, kind="ExternalInput")
    bqk = nc.dram_tensor("bqk", [128, 4], F32, kind="ExternalInput")
    beff = nc.dram_tensor("beff", [128, 2], F32, kind="ExternalInput")
    yd = nc.dram_tensor("y", [BS, C, S], F32, kind="ExternalOutput")

    with tile.TileContext(nc) as tc:
        ctx = _build_program(tc, xd, wqk, wv, wo, bqk, beff, yd)
        ctx.close()
    nc.compile()
    return nc


def _host_prep(x, w_qkv, b_qkv, w_out, b_out):
    import ml_dtypes

    x = np.asarray(x, dtype=np.float32)
    w_qkv = np.asarray(w_qkv, dtype=np.float32)
    b_qkv = np.asarray(b_qkv, dtype=np.float32)
    w_out = np.asarray(w_out, dtype=np.float32)
    b_out = np.asarray(b_out, dtype=np.float32)

    q_idx = np.concatenate([np.arange(h * 3 * DK, h * 3 * DK + DK) for h in range(NH)])
    k_idx = q_idx + DK
    v_idx = q_idx + 2 * DK

    bf16 = ml_dtypes.bfloat16
    x8 = np.ascontiguousarray(x.reshape(NCORES, BS, C, S))
    wqk_h = np.ascontiguousarray(
        w_qkv[:, np.concatenate([q_idx, k_idx])]).astype(bf16)
    wv_h = np.ascontiguousarray(w_qkv[:, v_idx]).astype(bf16)
    wo_h = np.ascontiguousarray(w_out).astype(bf16)
    bqk_h = np.ascontiguousarray(
        b_qkv[np.concatenate([q_idx, k_idx])].reshape(4, 128).T).astype(np.float32)
    b_v = b_qkv[v_idx]
    beff = b_out + (b_v.astype(np.float64) @ w_out.astype(np.float64)).astype(np.float32)
    beff_h = np.ascontiguousarray(beff.reshape(2, 128).T).astype(np.float32)
    return x8, wqk_h, wv_h, wo_h, bqk_h, beff_h


TRACE = False
LAST_RESULT = None


def kernel(x, w_qkv, b_qkv, w_out, b_out):
    global LAST_RESULT
    from concourse.bass_utils import run_bass_kernel_spmd

    if "nc" not in _CACHE:
        _CACHE["nc"] = _build_nc()
    nc = _CACHE["nc"]

    x8, wqk_h, wv_h, wo_h, bqk_h, beff_h = _host_prep(x, w_qkv, b_qkv, w_out, b_out)
    in_maps = [
        {"x": x8[c], "wqk": wqk_h, "wv": wv_h, "wo": wo_h,
         "bqk": bqk_h, "beff": beff_h}
        for c in range(NCORES)
    ]
    res = run_bass_kernel_spmd(nc, in_maps, core_ids=list(range(NCORES)), trace=TRACE)
    LAST_RESULT = res
    y = np.stack([res.results[c]["y"] for c in range(NCORES)])  # [8, BS, C, S]
    return np.ascontiguousarray(y.reshape(B, C, 32, 32)).astype(np.float32)


# revision 35
# speedup vs baseline: 1.0293x; 1.0293x over previous
"""AttentionBlock kernel for Trainium2, data-parallel over 8 NeuronCores.

Problem: x[16,256,32,32]; per sample (S=1024 tokens, C=256 channels):
  xs = x.reshape(C, S).T                      # [S, C]
  qkv = xs @ w_qkv + b_qkv                    # [S, 768] -> heads q,k,v (4 x 64)
  attn = softmax(q k^T / 8) ; o = attn @ v    # per head
  out = (concat_h o) @ w_out + b_out + xs     # [S, C] -> emitted as [C, S]

Device strategy (per core, 2 samples):
  Everything is computed in the TRANSPOSED layout so no on-chip transposes
  are needed:
    qkT[n, i]  = w_qkv[:, n].T @ x_nat          (x natural [C, S] == xs^T)
    S^T[j, i]  = K^T.T-contraction: lhsT=K^T[d, j], rhs=Q^T[d, i]
    P^T        = exp(S^T / 8)  (softmax w/o max-subtract; scores ~ N(0,1))
    O^T[d+1,i] = accum_j  V_ext[j, d|1].T @ P^T[j, i]  (ones col -> row sums)
    out^T[c,i] = w_out[m, c].T @ (O^T normalized)  + b_eff + x_nat
  b_qkv(v-part) commutes through softmax (rows sum to 1) so it is folded on
  the host into b_eff = b_out + b_v @ w_out.
"""

import numpy as np
from contextlib import ExitStack

B, C, S = 16, 256, 1024
NH, DK = 4, 64
NCORES = 8
BS = B // NCORES  # samples per core

_CACHE = {}


def _build_program(tc, xd, wqk, wv, wo, bqk, beff, yd):
    """Emit the per-core program into TileContext tc.

    xd   : [BS, 256, 1024] f32   input sample block (natural [C, S] layout)
    wqk  : [256, 512] bf16       qkv weights, columns permuted to [q0..q3 | k0..k3]
    wv   : [256, 256] bf16       v weights, columns [v0..v3]
    wo   : [256, 256] bf16       output projection
    bqk  : [128, 4] f32          q|k bias, bqk[p, t] = b[t*128 + p]
    beff : [128, 2] f32          b_out + b_v @ w_out, beff[p, ct] = b[ct*128 + p]
    yd   : [BS, 256, 1024] f32   output (natural [C, S] layout)
    """
    import concourse.bass as bass
    from concourse import mybir

    nc = tc.nc
    F32 = mybir.dt.float32
    BF16 = mybir.dt.bfloat16
    EXP = mybir.ActivationFunctionType.Exp
    IDENT = mybir.ActivationFunctionType.Identity
    MULT = mybir.AluOpType.mult
    ADD = mybir.AluOpType.add

    ctx = ExitStack()
    wts = ctx.enter_context(tc.tile_pool(name="wts", bufs=1))
    xfp = ctx.enter_context(tc.tile_pool(name="xfp", bufs=2))
    xbp = ctx.enter_context(tc.tile_pool(name="xbp", bufs=2))
    qkp = ctx.enter_context(tc.tile_pool(name="qkp", bufs=2))
    vvp = ctx.enter_context(tc.tile_pool(name="vvp", bufs=2))
    ptp = ctx.enter_context(tc.tile_pool(name="ptp", bufs=4))
    rrp = ctx.enter_context(tc.tile_pool(name="rrp", bufs=2))
    rbp = ctx.enter_context(tc.tile_pool(name="rbp", bufs=2))
    orp = ctx.enter_context(tc.tile_pool(name="orp", bufs=5))
    drp = ctx.enter_context(tc.tile_pool(name="drp", bufs=2, space="DRAM"))
    otp = ctx.enter_context(tc.tile_pool(name="otp", bufs=2))
    yop = ctx.enter_context(tc.tile_pool(name="yop", bufs=3))
    psb = ctx.enter_context(tc.tile_pool(name="psb", bufs=2, space="PSUM"))
    pso = ctx.enter_context(tc.tile_pool(name="pso", bufs=2, space="PSUM"))

    # --- weights / biases, loaded once ---
    wqk_sb = wts.tile([128, 2, 512], BF16)
    nc.sync.dma_start(wqk_sb[:], wqk[:, :].rearrange("(kc p) n -> p kc n", p=128))
    wv_sb = wts.tile([128, 2, 256], BF16)
    nc.sync.dma_start(wv_sb[:], wv[:, :].rearrange("(kc p) n -> p kc n", p=128))
    wo_sb = wts.tile([128, 2, 256], BF16)
    nc.sync.dma_start(wo_sb[:], wo[:, :].rearrange("(kc p) n -> p kc n", p=128))
    # Biases are staged through a DVE copy so downstream DVE consumers get a
    # same-engine dependency (no extra semaphore wait slot on the consumer —
    # walrus limits wait commands per instruction).
    bqk_st = wts.tile([128, 4], F32)
    nc.sync.dma_start(bqk_st[:], bqk[:, :])
    bqk_sb = wts.tile([128, 4], F32)
    nc.vector.tensor_copy(bqk_sb[:], bqk_st[:])
    beff_st = wts.tile([128, 2], F32)
    nc.sync.dma_start(beff_st[:], beff[:, :])
    beff_sb = wts.tile([128, 2], F32)
    nc.vector.tensor_copy(beff_sb[:], beff_st[:])

    def phase_load(s, st):
        """x load + bf16 cast (hoisted to the program start for all samples
        so the inter-sample pipeline never waits on the input DMA)"""
        x_sb = xfp.tile([128, 2, 1024], F32, name=f"x_{s}", tag="x")
        nc.sync.dma_start(x_sb[:], xd[s].rearrange("(kc p) i -> p kc i", p=128))
        xb_sb = xbp.tile([128, 2, 1024], BF16, name=f"xb_{s}", tag="xb")
        nc.vector.tensor_copy(xb_sb[:], x_sb[:])
        st.update(x_sb=x_sb, xb_sb=xb_sb)

    def phase_a(s, st):
        """qkT/V projections"""
        x_sb, xb_sb = st["x_sb"], st["xb_sb"]
        qk_sb = qkp.tile([128, 4, 1024], BF16, name=f"qk_{s}", tag="qk")
        v_sb = vvp.tile([128, 8, 4, 65], BF16, name=f"v_{s}", tag="v")

        def qk_tile(t):
            ps_qk = psb.tile([128, 1024], F32, tag="big", name=f"psqk_{s}_{t}")
            for kc in range(2):
                for ns in range(2):
                    nc.tensor.matmul(
                        ps_qk[:, ns * 512:(ns + 1) * 512],
                        lhsT=wqk_sb[:, kc, t * 128:(t + 1) * 128],
                        rhs=xb_sb[:, kc, ns * 512:(ns + 1) * 512],
                        start=(kc == 0), stop=(kc == 1),
                    )
            nc.vector.tensor_tensor(
                qk_sb[:, t, :], ps_qk[:],
                bqk_sb[:, t:t + 1].to_broadcast((128, 1024)), ADD)

        for t in range(4):
            qk_tile(t)
        nc.vector.memset(v_sb[:, :, :, 64:65], 1.0)
        for it in range(8):
            ps_v = psb.tile([128, 1024], F32, tag="big", name=f"psv_{s}_{it}")
            pv = ps_v[:, 0:256].rearrange("p (g d) -> p g d", g=4)
            for kc in range(2):
                nc.tensor.matmul(
                    pv,
                    lhsT=xb_sb[:, kc, it * 128:(it + 1) * 128],
                    rhs=wv_sb[:, kc, :],
                    start=(kc == 0), stop=(kc == 1),
                )
            nc.vector.tensor_copy(v_sb[:, it, :, 0:64], pv)
        st.update(qk_sb=qk_sb, v_sb=v_sb)

    def phase_b(s, st):
        """attention: heads sequential; pss bufs=2 gives one-jc lookahead for
        the exp/scores overlap; po bufs=2 overlaps adjacent heads; softmax
        normalization (reciprocal + gpsimd partition broadcast + multiply)
        is inlined per head and hides under the next head's attention."""
        qk_sb, v_sb = st["qk_sb"], st["v_sb"]
        ot_sb = otp.tile([128, 2, 1024], BF16, name=f"ot_{s}", tag="ot")
        srows = rrp.tile([33, 1024], F32, tag="sr", name=f"sr_{s}_pre")
        nc.vector.memset(srows[:], 1.0)
        oraw_tiles = []
        for h in range(4):
            bp = 64 * (h % 2)
            po = pso.tile([128, 1024], F32, tag="po", name=f"pso_{s}_{h}")
            for jc in range(8):
                pss = psb.tile([128, 1024], F32, tag="big", name=f"pss_{s}_{h}_{jc}")
                for ns in range(2):
                    nc.tensor.matmul(
                        pss[:, ns * 512:(ns + 1) * 512],
                        lhsT=qk_sb[bp:bp + 64, 2 + h // 2, jc * 128:(jc + 1) * 128],
                        rhs=qk_sb[bp:bp + 64, h // 2, ns * 512:(ns + 1) * 512],
                        start=True, stop=True,
                    )
                pt_sb = ptp.tile([128, 1024], BF16, tag="pt",
                                 name=f"pt_{s}_{h}_{jc}")
                nc.scalar.activation(pt_sb[:], pss[:], EXP,
                                     scale=float(DK) ** -0.5)
                for ns in range(2):
                    nc.tensor.matmul(
                        po[0:65, ns * 512:(ns + 1) * 512],
                        lhsT=v_sb[:, jc, h, :],
                        rhs=pt_sb[:, ns * 512:(ns + 1) * 512],
                        start=(jc == 0), stop=(jc == 7),
                    )
            # rows 0:64 = unnormalized P@V, row 64 = softmax denominator.
            # Evacuate PSUM promptly (frees the slot for head h+2); batch the
            # expensive DVE reciprocal per head pair; sums rows sit on
            # partitions {0,32} of a pair tile (32-aligned bases).
            oraw = orp.tile([64, 1024], BF16, tag="or", name=f"oraw_{s}_{h}")
            nc.vector.tensor_copy(oraw[:], po[0:64, :])
            nc.vector.tensor_copy(srows[32 * (h % 2):32 * (h % 2) + 1, :],
                                  po[64:65, :])
            oraw_tiles.append(oraw)
            if h % 2 == 1:
                srecip = rrp.tile([33, 1024], F32, tag="rc",
                                  name=f"srecip_{s}_{h // 2}")
                nc.vector.reciprocal(srecip[:], srows[:])
                sdram = drp.tile([2, 1024], F32, tag="sd",
                                 name=f"sdram_{s}_{h // 2}")
                for i in range(2):
                    nc.sync.dma_start(sdram[i:i + 1, :],
                                      srecip[32 * i:32 * i + 1, :])
                for i in range(2):
                    hh = 2 * (h // 2) + i
                    rb = rbp.tile([64, 1024], F32, tag="rb", name=f"rb_{s}_{hh}")
                    nc.sync.dma_start(
                        rb[:], sdram[i:i + 1, :].to_broadcast((64, 1024)))
                    nc.vector.tensor_tensor(
                        ot_sb[64 * i:64 * i + 64, h // 2, :],
                        oraw_tiles[hh][:], rb[:], MULT)
                srows = rrp.tile([33, 1024], F32, tag="sr",
                                 name=f"sr_{s}_{h // 2}")
                nc.vector.memset(srows[:], 1.0)
        st.update(ot_sb=ot_sb)

    def phase_out(s, st):
        """output projection, bias+residual, store."""
        x_sb, ot_sb = st["x_sb"], st["ot_sb"]
        for ct in range(2):
            ps_r = pso.tile([128, 1024], F32, tag="po", name=f"psr_{s}_{ct}")
            for mc in range(2):
                for ns in range(2):
                    nc.tensor.matmul(
                        ps_r[:, ns * 512:(ns + 1) * 512],
                        lhsT=wo_sb[:, mc, ct * 128:(ct + 1) * 128],
                        rhs=ot_sb[:, mc, ns * 512:(ns + 1) * 512],
                        start=(mc == 0), stop=(mc == 1),
                    )
            y_sb = yop.tile([128, 1024], F32, tag="yo", name=f"y_{s}_{ct}")
            nc.vector.scalar_tensor_tensor(
                y_sb[:], ps_r[:], beff_sb[:, ct:ct + 1], x_sb[:, ct, :],
                ADD, ADD)
            nc.sync.dma_start(yd[s, ct * 128:(ct + 1) * 128, :], y_sb[:])

    # software-pipelined emission: per-pair softmax normalization is emitted
    # inside phase_b (it executes during the other pair's attention); sample
    # s+1's projections are emitted before sample s's output projection
    states = [{} for _ in range(BS)]
    for s in range(BS):
        phase_load(s, states[s])
    for s in range(BS):
        phase_a(s, states[s])
        phase_b(s, states[s])
        phase_out(s, states[s])

    return ctx


def _build_nc():
    import concourse.tile as tile
    from concourse import bacc, mybir

    nc = bacc.Bacc(trn_type="TRN2")
    F32, BF16 = mybir.dt.float32, mybir.dt.bfloat16
    xd = nc.dram_tensor("x", [BS, C, S], F32, kind="ExternalInput")
    wqk = nc.dram_tensor("wqk", [C, 2 * NH * DK], BF16, kind="ExternalInput")
    wv = nc.dram_tensor("wv", [C, NH * DK], BF16, kind="ExternalInput")
    wo = nc.dram_tensor("wo", [C, C], BF16, kind="ExternalInput")
    bqk = nc.dram_tensor("bqk", [128, 4], F32, kind="ExternalInput")
    beff = nc.dram_tensor("beff", [128, 2], F32, kind="ExternalInput")
    yd = nc.dram_tensor("y", [BS, C, S], F32, kind="ExternalOutput")

    with tile.TileContext(nc) as tc:
        ctx = _build_program(tc, xd, wqk, wv, wo, bqk, beff, yd)
        ctx.close()
    nc.compile()
    return nc


def _host_prep(x, w_qkv, b_qkv, w_out, b_out):
    import ml_dtypes

    x = np.asarray(x, dtype=np.float32)
    w_qkv = np.asarray(w_qkv, dtype=np.float32)
    b_qkv = np.asarray(b_qkv, dtype=np.float32)
    w_out = np.asarray(w_out, dtype=np.float32)
    b_out = np.asarray(b_out, dtype=np.float32)

    q_idx = np.concatenate([np.arange(h * 3 * DK, h * 3 * DK + DK) for h in range(NH)])
    k_idx = q_idx + DK
    v_idx = q_idx + 2 * DK

    bf16 = ml_dtypes.bfloat16
    x8 = np.ascontiguousarray(x.reshape(NCORES, BS, C, S))
    wqk_h = np.ascontiguousarray(
        w_qkv[:, np.concatenate([q_idx, k_idx])]).astype(bf16)
    wv_h = np.ascontiguousarray(w_qkv[:, v_idx]).astype(bf16)
    wo_h = np.ascontiguousarray(w_out).astype(bf16)
    bqk_h = np.ascontiguousarray(
        b_qkv[np.concatenate([q_idx, k_idx])].reshape(4, 128).T).astype(np.float32)
    b_v = b_qkv[v_idx]
    beff = b_out + (b_v.astype(np.float64) @ w_out.astype(np.float64)).astype(np.float32)
    beff_h = np.ascontiguousarray(beff.reshape(2, 128).T).astype(np.float32)
    return x8, wqk_h, wv_h, wo_h, bqk_h, beff_h


TRACE = False
LAST_RESULT = None


def kernel(x, w_qkv, b_qkv, w_out, b_out):
    global LAST_RESULT
    from concourse.bass_utils import run_bass_kernel_spmd

    if "nc" not in _CACHE:
        _CACHE["nc"] = _build_nc()
    nc = _CACHE["nc"]

    x8, wqk_h, wv_h, wo_h, bqk_h, beff_h = _host_prep(x, w_qkv, b_qkv, w_out, b_out)
    in_maps = [
        {"x": x8[c], "wqk": wqk_h, "wv": wv_h, "wo": wo_h,
         "bqk": bqk_h, "beff": beff_h}
        for c in range(NCORES)
    ]
    res = run_bass_kernel_spmd(nc, in_maps, core_ids=list(range(NCORES)), trace=TRACE)
    LAST_RESULT = res
    y = np.stack([res.results[c]["y"] for c in range(NCORES)])  # [8, BS, C, S]
    return np.ascontiguousarray(y.reshape(B, C, 32, 32)).astype(np.float32)


# revision 36
# speedup vs baseline: 1.1829x; 1.1492x over previous
"""AttentionBlock kernel for Trainium2, data-parallel over 8 NeuronCores.

Problem: x[16,256,32,32]; per sample (S=1024 tokens, C=256 channels):
  xs = x.reshape(C, S).T                      # [S, C]
  qkv = xs @ w_qkv + b_qkv                    # [S, 768] -> heads q,k,v (4 x 64)
  attn = softmax(q k^T / 8) ; o = attn @ v    # per head
  out = (concat_h o) @ w_out + b_out + xs     # [S, C] -> emitted as [C, S]

Device strategy (per core, 2 samples):
  Everything is computed in the TRANSPOSED layout so no on-chip transposes
  are needed:
    qkT[n, i]  = w_qkv[:, n].T @ x_nat          (x natural [C, S] == xs^T)
    S^T[j, i]  = K^T.T-contraction: lhsT=K^T[d, j], rhs=Q^T[d, i]
    P^T        = exp(S^T / 8)  (softmax w/o max-subtract; scores ~ N(0,1))
    O^T[d+1,i] = accum_j  V_ext[j, d|1].T @ P^T[j, i]  (ones col -> row sums)
    out^T[c,i] = w_out[m, c].T @ (O^T normalized)  + b_eff + x_nat
  b_qkv(v-part) commutes through softmax (rows sum to 1) so it is folded on
  the host into b_eff = b_out + b_v @ w_out.
"""

import numpy as np
from contextlib import ExitStack

B, C, S = 16, 256, 1024
NH, DK = 4, 64
NCORES = 8
BS = B // NCORES  # samples per core

_CACHE = {}


def _build_program(tc, xd, wqk, wv, wo, bqk, beff, yd):
    """Emit the per-core program into TileContext tc.

    xd   : [BS, 256, 1024] f32   input sample block (natural [C, S] layout)
    wqk  : [256, 512] bf16       qkv weights, columns permuted to [q0..q3 | k0..k3]
    wv   : [256, 256] bf16       v weights, columns [v0..v3]
    wo   : [256, 256] bf16       output projection
    bqk  : [128, 4] f32          q|k bias, bqk[p, t] = b[t*128 + p]
    beff : [128, 2] f32          b_out + b_v @ w_out, beff[p, ct] = b[ct*128 + p]
    yd   : [BS, 256, 1024] f32   output (natural [C, S] layout)
    """
    import concourse.bass as bass
    from concourse import mybir

    nc = tc.nc
    F32 = mybir.dt.float32
    BF16 = mybir.dt.bfloat16
    EXP = mybir.ActivationFunctionType.Exp
    IDENT = mybir.ActivationFunctionType.Identity
    MULT = mybir.AluOpType.mult
    ADD = mybir.AluOpType.add

    ctx = ExitStack()
    wts = ctx.enter_context(tc.tile_pool(name="wts", bufs=1))
    xfp = ctx.enter_context(tc.tile_pool(name="xfp", bufs=2))
    xbp = ctx.enter_context(tc.tile_pool(name="xbp", bufs=2))
    qkp = ctx.enter_context(tc.tile_pool(name="qkp", bufs=2))
    vvp = ctx.enter_context(tc.tile_pool(name="vvp", bufs=2))
    ptp = ctx.enter_context(tc.tile_pool(name="ptp", bufs=4))
    rrp = ctx.enter_context(tc.tile_pool(name="rrp", bufs=2))
    rbp = ctx.enter_context(tc.tile_pool(name="rbp", bufs=2))
    orp = ctx.enter_context(tc.tile_pool(name="orp", bufs=5))
    drp = ctx.enter_context(tc.tile_pool(name="drp", bufs=2, space="DRAM"))
    otp = ctx.enter_context(tc.tile_pool(name="otp", bufs=2))
    yop = ctx.enter_context(tc.tile_pool(name="yop", bufs=3))
    psb = ctx.enter_context(tc.tile_pool(name="psb", bufs=2, space="PSUM"))
    pso = ctx.enter_context(tc.tile_pool(name="pso", bufs=2, space="PSUM"))

    # --- weights / biases, loaded once ---
    wqk_sb = wts.tile([128, 2, 512], BF16)
    nc.sync.dma_start(wqk_sb[:], wqk[:, :].rearrange("(kc p) n -> p kc n", p=128))
    wv_sb = wts.tile([128, 2, 256], BF16)
    nc.sync.dma_start(wv_sb[:], wv[:, :].rearrange("(kc p) n -> p kc n", p=128))
    wo_sb = wts.tile([128, 2, 256], BF16)
    nc.sync.dma_start(wo_sb[:], wo[:, :].rearrange("(kc p) n -> p kc n", p=128))
    # Biases are staged through a DVE copy so downstream DVE consumers get a
    # same-engine dependency (no extra semaphore wait slot on the consumer —
    # walrus limits wait commands per instruction).
    bqk_st = wts.tile([128, 4], F32)
    nc.sync.dma_start(bqk_st[:], bqk[:, :])
    bqk_sb = wts.tile([128, 4], F32)
    nc.vector.tensor_copy(bqk_sb[:], bqk_st[:])
    beff_st = wts.tile([128, 2], F32)
    nc.sync.dma_start(beff_st[:], beff[:, :])
    beff_sb = wts.tile([128, 2], F32)
    nc.vector.tensor_copy(beff_sb[:], beff_st[:])

    def phase_load(s, st):
        """x load + bf16 cast (hoisted to the program start for all samples
        so the inter-sample pipeline never waits on the input DMA)"""
        x_sb = xfp.tile([128, 2, 1024], F32, name=f"x_{s}", tag="x")
        nc.sync.dma_start(x_sb[:], xd[s].rearrange("(kc p) i -> p kc i", p=128))
        xb_sb = xbp.tile([128, 2, 1024], BF16, name=f"xb_{s}", tag="xb")
        nc.vector.tensor_copy(xb_sb[:], x_sb[:])
        st.update(x_sb=x_sb, xb_sb=xb_sb)

    def phase_a(s, st):
        """qkT/V projections"""
        x_sb, xb_sb = st["x_sb"], st["xb_sb"]
        qk_sb = qkp.tile([128, 4, 1024], BF16, name=f"qk_{s}", tag="qk")
        v_sb = vvp.tile([128, 8, 4, 65], BF16, name=f"v_{s}", tag="v")

        def qk_tile(t):
            ps_qk = psb.tile([128, 1024], F32, tag="big", name=f"psqk_{s}_{t}")
            for kc in range(2):
                for ns in range(2):
                    nc.tensor.matmul(
                        ps_qk[:, ns * 512:(ns + 1) * 512],
                        lhsT=wqk_sb[:, kc, t * 128:(t + 1) * 128],
                        rhs=xb_sb[:, kc, ns * 512:(ns + 1) * 512],
                        start=(kc == 0), stop=(kc == 1),
                    )
            nc.vector.tensor_tensor(
                qk_sb[:, t, :], ps_qk[:],
                bqk_sb[:, t:t + 1].to_broadcast((128, 1024)), ADD)

        for t in range(4):
            qk_tile(t)
        nc.vector.memset(v_sb[:, :, :, 64:65], 1.0)
        for it in range(8):
            ps_v = psb.tile([128, 1024], F32, tag="big", name=f"psv_{s}_{it}")
            pv = ps_v[:, 0:256].rearrange("p (g d) -> p g d", g=4)
            for kc in range(2):
                nc.tensor.matmul(
                    pv,
                    lhsT=xb_sb[:, kc, it * 128:(it + 1) * 128],
                    rhs=wv_sb[:, kc, :],
                    start=(kc == 0), stop=(kc == 1),
                )
            nc.vector.tensor_copy(v_sb[:, it, :, 0:64], pv)
        st.update(qk_sb=qk_sb, v_sb=v_sb)

    def phase_b(s, st):
        """attention: heads sequential; pss bufs=2 gives one-jc lookahead for
        the exp/scores overlap; po bufs=2 overlaps adjacent heads; softmax
        normalization (reciprocal + gpsimd partition broadcast + multiply)
        is inlined per head and hides under the next head's attention."""
        qk_sb, v_sb = st["qk_sb"], st["v_sb"]
        ot_sb = otp.tile([128, 2, 1024], BF16, name=f"ot_{s}", tag="ot")
        for h in range(4):
            bp = 64 * (h % 2)
            po = pso.tile([128, 1024], F32, tag="po", name=f"pso_{s}_{h}")
            for jc in range(8):
                pss = psb.tile([128, 1024], F32, tag="big", name=f"pss_{s}_{h}_{jc}")
                for ns in range(2):
                    nc.tensor.matmul(
                        pss[:, ns * 512:(ns + 1) * 512],
                        lhsT=qk_sb[bp:bp + 64, 2 + h // 2, jc * 128:(jc + 1) * 128],
                        rhs=qk_sb[bp:bp + 64, h // 2, ns * 512:(ns + 1) * 512],
                        start=True, stop=True,
                    )
                pt_sb = ptp.tile([128, 1024], BF16, tag="pt",
                                 name=f"pt_{s}_{h}_{jc}")
                nc.scalar.activation(pt_sb[:], pss[:], EXP,
                                     scale=float(DK) ** -0.5)
                for ns in range(2):
                    nc.tensor.matmul(
                        po[0:65, ns * 512:(ns + 1) * 512],
                        lhsT=v_sb[:, jc, h, :],
                        rhs=pt_sb[:, ns * 512:(ns + 1) * 512],
                        start=(jc == 0), stop=(jc == 7),
                    )
            # rows 0:64 = unnormalized P@V, row 64 = softmax denominator
            rrow = rrp.tile([1, 1024], F32, tag="rr", name=f"rr_{s}_{h}")
            nc.vector.reciprocal(rrow[:], po[64:65, :])
            rb = rbp.tile([64, 1024], F32, tag="rb", name=f"rb_{s}_{h}")
            nc.gpsimd.partition_broadcast(rb[:], rrow[:], channels=64)
            nc.vector.tensor_tensor(
                ot_sb[bp:bp + 64, h // 2, :], po[0:64, :], rb[:], MULT)
        st.update(ot_sb=ot_sb)

    def phase_out(s, st):
        """output projection, bias+residual, store."""
        x_sb, ot_sb = st["x_sb"], st["ot_sb"]
        for ct in range(2):
            ps_r = pso.tile([128, 1024], F32, tag="po", name=f"psr_{s}_{ct}")
            for mc in range(2):
                for ns in range(2):
                    nc.tensor.matmul(
                        ps_r[:, ns * 512:(ns + 1) * 512],
                        lhsT=wo_sb[:, mc, ct * 128:(ct + 1) * 128],
                        rhs=ot_sb[:, mc, ns * 512:(ns + 1) * 512],
                        start=(mc == 0), stop=(mc == 1),
                    )
            y_sb = yop.tile([128, 1024], F32, tag="yo", name=f"y_{s}_{ct}")
            nc.vector.scalar_tensor_tensor(
                y_sb[:], ps_r[:], beff_sb[:, ct:ct + 1], x_sb[:, ct, :],
                ADD, ADD)
            nc.sync.dma_start(yd[s, ct * 128:(ct + 1) * 128, :], y_sb[:])

    # software-pipelined emission: per-pair softmax normalization is emitted
    # inside phase_b (it executes during the other pair's attention); sample
    # s+1's projections are emitted before sample s's output projection
    states = [{} for _ in range(BS)]
    for s in range(BS):
        phase_load(s, states[s])
    for s in range(BS):
        phase_a(s, states[s])
        phase_b(s, states[s])
        phase_out(s, states[s])

    return ctx


def _build_nc():
    import concourse.tile as tile
    from concourse import bacc, mybir

    nc = bacc.Bacc(trn_type="TRN2")
    F32, BF16 = mybir.dt.float32, mybir.dt.bfloat16
    xd = nc.dram_tensor("x", [BS, C, S], F32, kind="ExternalInput")
    wqk = nc.dram_tensor("wqk", [C, 2 * NH * DK], BF16, kind="ExternalInput")
    wv = nc.dram_tensor("wv", [C, NH * DK], BF16, kind="ExternalInput")
    wo = nc.dram_tensor("wo", [C, C], BF16, kind="ExternalInput")
    bqk = nc.dram_tensor("bqk", [128, 4], F32, kind="ExternalInput")
    beff = nc.dram_tensor("beff", [128, 2], F32, kind="ExternalInput")
    yd = nc.dram_tensor("y", [BS, C, S], F32, kind="ExternalOutput")

    with tile.TileContext(nc) as tc:
        ctx = _build_program(tc, xd, wqk, wv, wo, bqk, beff, yd)
        ctx.close()
    nc.compile()
    return nc


def _host_prep(x, w_qkv, b_qkv, w_out, b_out):
    import ml_dtypes

    x = np.asarray(x, dtype=np.float32)
    w_qkv = np.asarray(w_qkv, dtype=np.float32)
    b_qkv = np.asarray(b_qkv, dtype=np.float32)
    w_out = np.asarray(w_out, dtype=np.float32)
    b_out = np.asarray(b_out, dtype=np.float32)

    q_idx = np.concatenate([np.arange(h * 3 * DK, h * 3 * DK + DK) for h in range(NH)])
    k_idx = q_idx + DK
    v_idx = q_idx + 2 * DK

    bf16 = ml_dtypes.bfloat16
    x8 = np.ascontiguousarray(x.reshape(NCORES, BS, C, S))
    wqk_h = np.ascontiguousarray(
        w_qkv[:, np.concatenate([q_idx, k_idx])]).astype(bf16)
    wv_h = np.ascontiguousarray(w_qkv[:, v_idx]).astype(bf16)
    wo_h = np.ascontiguousarray(w_out).astype(bf16)
    bqk_h = np.ascontiguousarray(
        b_qkv[np.concatenate([q_idx, k_idx])].reshape(4, 128).T).astype(np.float32)
    b_v = b_qkv[v_idx]
    beff = b_out + (b_v.astype(np.float64) @ w_out.astype(np.float64)).astype(np.float32)
    beff_h = np.ascontiguousarray(beff.reshape(2, 128).T).astype(np.float32)
    return x8, wqk_h, wv_h, wo_h, bqk_h, beff_h


TRACE = False
LAST_RESULT = None


def kernel(x, w_qkv, b_qkv, w_out, b_out):
    global LAST_RESULT
    from concourse.bass_utils import run_bass_kernel_spmd

    if "nc" not in _CACHE:
        _CACHE["nc"] = _build_nc()
    nc = _CACHE["nc"]

    x8, wqk_h, wv_h, wo_h, bqk_h, beff_h = _host_prep(x, w_qkv, b_qkv, w_out, b_out)
    in_maps = [
        {"x": x8[c], "wqk": wqk_h, "wv": wv_h, "wo": wo_h,
         "bqk": bqk_h, "beff": beff_h}
        for c in range(NCORES)
    ]
    res = run_bass_kernel_spmd(nc, in_maps, core_ids=list(range(NCORES)), trace=TRACE)
    LAST_RESULT = res
    y = np.stack([res.results[c]["y"] for c in range(NCORES)])  # [8, BS, C, S]
    return np.ascontiguousarray(y.reshape(B, C, 32, 32)).astype(np.float32)
